# revision 4
# baseline (speedup 1.0000x reference)
"""Trainium2 Bass kernel for NeuralDecisionTree (histogram_binning).

Math: out[b,c] = mean_t sum_l (prod_f h[b,t,f,bit_f(l)]) * score[l,c] with
h[...,0] = x, h[...,1] = 2x - cut_f  (D=1 -> W=[1,2], bias=[0,-cut]).

The 4096-leaf weight vector is kron(A6, B6) of two 64-leaf halves (features
0-5 -> i, 6-11 -> j, l = i*64 + j), each half kron(P8, Q8) of two 8-wide
3-feature factors.  The host precomputes the four 8-wide factors (PA, QA,
PB, QB) in f16; the device builds A6 = PA x QA and B6 = PB x QB (the
dominant elementwise work, split across DVE and GpSimd), reduces each
sample to a 64x64 second-moment matrix on the TensorEngine (psum partition
p = leaf%128, col = leaf//128), and contracts with leaf_score in fp8e4m3
DoubleRow matmuls (leaf_score stationary, psum out = [class, sample]).

Sharding: leaf_score is sharded by class columns (125 per core); x is
replicated and stage 1 recomputed per core.

DMA: factors upload in 4 bs-chunks; chunk 0 goes through a SWDGE
prepare/trigger gather (skips the HWDGE + DGE-start latency) so the outer
products start early, chunks 1-3 + scores pipeline on the SP HWDGE queue.
The output is written by a pre-prepared SWDGE kv_writeback triggered after
the final psum->sbuf copy, cutting the output-DMA tail to the transfer +
semaphore time.  A dummy-matmul warmup stream holds the PE p-state at full
clock for the stage-1/stage-2 bursts.
"""

import numpy as np
import ml_dtypes

B, T, H = 16, 512, 12
NCORES = 8
C = 1000
CS = C // NCORES          # 125 classes per core
SP = 4                    # t = p*4 + s
BS = B * SP               # 64 (b-major: bs = b*4 + s)
NCH = 4                   # factor upload chunks (16 bs = 4 samples each)
CW = BS // NCH            # 16 bs per chunk
KCH = 32                  # 128-leaf chunks
NPAD = 128                # padded class cols in the fp8 score tile
CHB = 4 * 8 * CW          # factor cols per chunk (512)
DVE_Q = 9                 # bs-cols of each B6 chunk built on DVE (rest: Pool)
N_WARM0 = 50              # PE warmup matmuls before stage-1
N_WARMG = 16              # PE keep-warm matmuls between chunk bursts
SWDGE_IN = False           # chunk-0 factors via SWDGE gather
SWDGE_OUT = False          # output via SWDGE kv_writeback


def _build_nc():
    import concourse.bass as bass
    import concourse.bacc as bacc
    import concourse.mybir as mybir
    from concourse import tile

    f32 = mybir.dt.float32
    f16 = mybir.dt.float16
    f8 = mybir.dt.float8e4
    i16 = mybir.dt.int16
    i32 = mybir.dt.int32
    Act = mybir.ActivationFunctionType

    nc = bacc.Bacc(None, target_bir_lowering=False, debug=False)

    # factors: [p, chunk, fac(PA,QA,PB,QB), 8, q]  (q = bs within chunk)
    fx_d = nc.dram_tensor("fx", [128, NCH * CHB], f16, kind="ExternalInput")
    s_d = nc.dram_tensor("s", [128, KCH * NPAD], f8, kind="ExternalInput")
    # out[b, class] (class padded to 128); kv_writeback writes whole rows
    o_d = nc.dram_tensor("o", [B, 128], f32, kind="ExternalOutput")

    with tile.TileContext(nc) as tc:
        with (
            tc.tile_pool(name="io", bufs=1) as io,
            tc.tile_pool(name="work", bufs=1) as work,
            tc.tile_pool(name="psum", bufs=1, space="PSUM") as psum,
        ):
            FX = io.tile([128, NCH * CHB], f16)
            SC = io.tile([128, KCH * NPAD], f8)
            osb = work.tile([128, B], f32)  # [class(pad 128), b]

            if SWDGE_IN:
                gidx = work.tile([16, 8], i16)
                nc.gpsimd.iota(gidx[:], [[16, 8]], base=0, channel_multiplier=1)
                gsem = nc.alloc_semaphore(name="g0_dma")
                nc.gpsimd.dma_gather(
                    FX[:, 0:CHB].rearrange("p (g e) -> p g e", g=1),
                    fx_d[:, 0:CHB],
                    gidx[:],
                    num_idxs=128,
                    num_idxs_reg=128,
                    elem_size=CHB,
                    elem_step=NCH * CHB,
                    prepare_only=True,
                    sem=gsem,
                )
                nc.gpsimd.trigger_dma(count=None)
            else:
                nc.sync.dma_start(FX[:, 0:CHB], fx_d[:, 0:CHB])

            if SWDGE_OUT:
                nc.vector.memzero(osb[:])
                widx = work.tile([128, B], i32)
                nc.vector.memzero(widx[:])
                wsem = nc.alloc_semaphore(name="o_dma")
                nc.gpsimd.kv_writeback(
                    o_d[:].rearrange("b (dhi dho ctx) -> b dhi dho ctx", dhi=128,
                                     dho=1, ctx=1),
                    osb[:].rearrange("p (dho b ncn) -> p dho b ncn", dho=1,
                                     b=B, ncn=1),
                    widx[:],
                    prepare_only=True,
                    sem=wsem,
                )

            for c in range(1, NCH):
                sl = slice(c * CHB, (c + 1) * CHB)
                nc.sync.dma_start(FX[:, sl], fx_d[:, sl])
            nc.sync.dma_start(SC[:], s_d[:])

            FXv = FX[:].rearrange(
                "p (c f e q) -> p c f e q", c=NCH, f=4, e=8, q=CW
            )

            A6 = work.tile([128, 64 * BS], f16)
            B6 = work.tile([128, 64 * BS], f16)
            # [p, hi, lo, chunk, q]
            A6v = A6[:].rearrange(
                "p (hi lo c q) -> p hi lo c q", hi=8, lo=8, c=NCH, q=CW
            )
            B6v = B6[:].rearrange(
                "p (hi lo c q) -> p hi lo c q", hi=8, lo=8, c=NCH, q=CW
            )

            T8 = work.tile([128, KCH * B], f8)  # cols: k*16 + b
            T8v = T8[:].rearrange("p (k b) -> p k b", k=KCH, b=B)

            # PE warmup stream: junk matmuls to ramp/hold the p-state
            dw = work.tile([128, 64], f16)
            nc.vector.memzero(dw[:])
            dp = psum.tile([64, 64], f32, tag="warm")

            def warm(n):
                for _ in range(n):
                    nc.tensor.matmul(
                        dp[:], dw[:], dw[:], start=True, stop=True,
                        skip_group_check=True,
                    )

            warm(N_WARM0)

            def outer(eng, out_v, a_v, b_v, w):
                eng.tensor_mul(
                    out_v,
                    a_v.unsqueeze(2).broadcast_to((128, 8, 8, w)),
                    b_v.unsqueeze(1).broadcast_to((128, 8, 8, w)),
                )

            for c in range(NCH):
                # device kron: A6 = PA x QA (DVE), B6 = PB x QB (DVE+Pool)
                outer(nc.vector, A6v[:, :, :, c, :], FXv[:, c, 0], FXv[:, c, 1], CW)
                outer(
                    nc.vector,
                    B6v[:, :, :, c, :DVE_Q],
                    FXv[:, c, 2, :, :DVE_Q],
                    FXv[:, c, 3, :, :DVE_Q],
                    DVE_Q,
                )
                outer(
                    nc.gpsimd,
                    B6v[:, :, :, c, DVE_Q:],
                    FXv[:, c, 2, :, DVE_Q:],
                    FXv[:, c, 3, :, DVE_Q:],
                    CW - DVE_Q,
                )

                # stage 1: per (sample, parity) accumulate over s
                # psum pt[p, lb*32 + k], p = j + 64*(i&1), k = i>>1
                pt = psum.tile([128, 4 * KCH], f32, tag=f"ps{c}")
                Bc = B6v[:, :, :, c, :].rearrange("p hi lo q -> p (hi lo) q")
                for lb in range(4):
                    col = slice(lb * KCH, (lb + 1) * KCH)
                    for s in range(SP):
                        q = lb * SP + s
                        lhsT = Bc[:, :, q]
                        nc.tensor.matmul(
                            pt[0:64, col], lhsT, A6v[:, :, 0::2, c, q],
                            start=(s == 0), stop=(s == SP - 1),
                            skip_group_check=True,
                        )
                        nc.tensor.matmul(
                            pt[64:128, col], lhsT, A6v[:, :, 1::2, c, q],
                            start=(s == 0), stop=(s == SP - 1),
                            tile_position=(0, 64),
                            skip_group_check=True,
                        )

                # psum -> T8 (fp8e4) with the t-mean scale
                nc.scalar.activation(
                    T8v[:, :, 4 * c:4 * c + 4],
                    pt[:].rearrange("p (lb k) -> p k lb", lb=4, k=KCH),
                    Act.Copy,
                    scale=1.0 / T,
                )
                if c < NCH - 1:
                    warm(N_WARMG)

            # stage 2: fp8 DoubleRow, scores stationary -> psum [class, b]
            SCv = SC[:].rearrange("p (k n) -> p k n", k=KCH, n=NPAD)
            op = psum.tile([CS, B], f32, tag="out")
            for c in range(KCH // 2):
                nc.tensor.matmul(
                    op[:],
                    SCv[:, 2 * c:2 * c + 2, :CS],
                    T8v[:, 2 * c:2 * c + 2, :],
                    start=(c == 0), stop=(c == KCH // 2 - 1),
                    perf_mode=mybir.MatmulPerfMode.DoubleRow,
                    skip_group_check=True,
                )
            nc.scalar.activation(osb[0:CS, :], op[:], Act.Copy)
            if SWDGE_OUT:
                nc.gpsimd.trigger_dma(count=None)
            else:
                nc.sync.dma_start(
                    o_d[:].rearrange("b c -> c b")[0:CS, :], osb[0:CS, :]
                )

    nc.compile()
    return nc


_NC_CACHE = None


def _get_nc():
    global _NC_CACHE
    if _NC_CACHE is None:
        _NC_CACHE = _build_nc()
    return _NC_CACHE


def make_in_maps(x, cuts, leaf_score):
    xl = np.ascontiguousarray(x[-1], dtype=np.float32)  # [B, T, H]
    cut = cuts[:, 0].astype(np.float32)                 # [H]
    # t = p*4 + s ;  xp[p, b, s, f]
    xp = xl.reshape(B, 128, SP, H).transpose(1, 0, 2, 3)
    h = np.stack([xp, 2.0 * xp - cut], axis=-1)         # [p, b, s, f, 2]

    def k3(f0):
        a = h[..., f0, :].astype(np.float16)
        b_ = h[..., f0 + 1, :].astype(np.float16)
        c = h[..., f0 + 2, :].astype(np.float16)
        ab = (a[..., :, None] * b_[..., None, :]).reshape(128, B, SP, 4)
        return (ab[..., :, None].astype(np.float16)
                * c[..., None, :]).astype(np.float16).reshape(128, B, SP, 8)

    # fac[p, b, s, fac, 8]; repack to [p, chunk, fac, 8, q] (q = lb*4 + s)
    fac = np.stack([k3(0), k3(3), k3(6), k3(9)], axis=3)
    fx = (fac.reshape(128, NCH, 4, SP, 4, 8)
          .transpose(0, 1, 4, 5, 2, 3)                  # [p, c, fac, 8, lb, s]
          .reshape(128, NCH * CHB))
    fx = np.ascontiguousarray(fx, dtype=np.float16)

    # scores: fp8e4m3 [p, k, n] with n padded to 128
    s8 = np.zeros((NCORES, 128, KCH, NPAD), dtype=ml_dtypes.float8_e4m3)
    sl = leaf_score.astype(np.float32).reshape(KCH, 128, C)
    for m in range(NCORES):
        s8[m, :, :, :CS] = (
            sl[:, :, m * CS:(m + 1) * CS].transpose(1, 0, 2)
            .astype(ml_dtypes.float8_e4m3)
        )
    return [
        {"fx": fx, "s": np.ascontiguousarray(s8[m].reshape(128, KCH * NPAD))}
        for m in range(NCORES)
    ]


def kernel(x, cuts, leaf_score):
    from concourse import bass_utils

    nc = _get_nc()
    in_maps = make_in_maps(x, cuts, leaf_score)
    res = bass_utils.run_bass_kernel_spmd(nc, in_maps, list(range(NCORES)))
    out = np.concatenate(
        [res.results[m]["o"][:, :CS] for m in range(NCORES)], axis=1
    )
    return out.astype(np.float32)


# revision 13
# speedup vs baseline: 1.0397x; 1.0397x over previous
"""Trainium2 Bass kernel for NeuralDecisionTree (histogram_binning).

Math: out[b,c] = mean_t sum_l (prod_f h[b,t,f,bit_f(l)]) * score[l,c] with
h[...,0] = x, h[...,1] = 2x - cut_f  (D=1 -> W=[1,2], bias=[0,-cut]).

The 4096-leaf weight vector is kron(A6, B6) of two 64-leaf halves (features
0-5 -> i, 6-11 -> j, l = i*64 + j), each half kron(P8, Q8) of two 8-wide
3-feature factors.  The host precomputes the four 8-wide factors (PA, QA,
PB, QB) in f16; the device builds A6 = PA x QA and B6 = PB x QB (the
dominant elementwise work, split across DVE and GpSimd), reduces each
sample to a 64x64 second-moment matrix on the TensorEngine (psum partition
p = leaf%128, col = leaf//128), and contracts with leaf_score in fp8e4m3
DoubleRow matmuls (leaf_score stationary, psum out = [class, sample]).

Sharding: leaf_score is sharded by class columns (125 per core); x is
replicated and stage 1 recomputed per core.

Pipeline: factors upload in unequal sample-chunks (small first chunk for an
early DVE start, 1-sample last chunk to shorten the serial tail), scores
last on the same SP queue; per chunk the stage-1 matmuls and psum->sbuf
fp8 copies trail the DVE/Pool krons; output is written class-major
([128, B] f32) so the final DMA is a single cheap row-per-partition
transfer.  A dummy-matmul warmup stream holds the PE p-state at full clock
for the stage-1/stage-2 bursts.
"""

import numpy as np
import ml_dtypes

B, T, H = 16, 512, 12
NCORES = 8
C = 1000
CS = C // NCORES          # 125 classes per core
SP = 4                    # t = p*4 + s
BS = B * SP               # 64 (b-major: bs = b*4 + s)
KCH = 32                  # 128-leaf chunks
NPAD = 128                # padded class cols in the fp8 score tile
CHUNKS = (3, 4, 4, 4, 1)  # samples per factor-upload chunk
SEB = 4 * 8 * SP          # factor cols per sample (128)
N_WARM0 = 50              # PE warmup matmuls before stage-1
N_WARMG = 16              # PE keep-warm matmuls between chunk bursts


def _dve_q(s):
    """bs-cols of a chunk's B6 built on DVE (rest on GpSimd), balancing
    DVE (0.52 ns/el + 60/op) against GpSimd (1.98 ns/el + 95/op)."""
    return max(0, min(4 * s, round((149.5 * s - 10) / 64)))


def _build_nc():
    import concourse.bass as bass
    import concourse.bacc as bacc
    import concourse.mybir as mybir
    from concourse import tile

    f32 = mybir.dt.float32
    f16 = mybir.dt.float16
    f8 = mybir.dt.float8e4
    Act = mybir.ActivationFunctionType

    nc = bacc.Bacc(None, target_bir_lowering=False, debug=False)

    # factors: [p, sample, fac(PA,QA,PB,QB), 8, s]
    fx_d = nc.dram_tensor("fx", [128, B * SEB], f16, kind="ExternalInput")
    s_d = nc.dram_tensor("s", [128, KCH * NPAD], f8, kind="ExternalInput")
    o_d = nc.dram_tensor("o", [B, CS], f32, kind="ExternalOutput")

    with tile.TileContext(nc) as tc:
        with (
            tc.tile_pool(name="io", bufs=1) as io,
            tc.tile_pool(name="work", bufs=1) as work,
            tc.tile_pool(name="psum", bufs=1, space="PSUM") as psum,
        ):
            FX = io.tile([128, B * SEB], f16)
            SC = io.tile([128, KCH * NPAD], f8)
            b0 = 0
            for ns in CHUNKS:
                sl = slice(b0 * SEB, (b0 + ns) * SEB)
                nc.sync.dma_start(FX[:, sl], fx_d[:, sl])
                b0 += ns
            nc.sync.dma_start(SC[:], s_d[:])

            A6 = work.tile([128, 64 * BS], f16)
            B6 = work.tile([128, 64 * BS], f16)
            # [p, hi, lo, bs]  (bs = b*4 + s)
            A6v = A6[:].rearrange("p (hi lo q) -> p hi lo q", hi=8, lo=8, q=BS)
            B6v = B6[:].rearrange("p (hi lo q) -> p hi lo q", hi=8, lo=8, q=BS)

            T8 = work.tile([128, KCH * B], f8)  # cols: k*16 + b
            T8v = T8[:].rearrange("p (k b) -> p k b", k=KCH, b=B)

            # PE warmup stream: junk matmuls to ramp/hold the p-state
            dw = work.tile([128, 64], f16)
            nc.vector.memzero(dw[:])
            dp = psum.tile([64, 64], f32, tag="warm")

            def warm(n):
                for _ in range(n):
                    nc.tensor.matmul(
                        dp[:], dw[:], dw[:], start=True, stop=True,
                        skip_group_check=True,
                    )

            warm(N_WARM0)

            def outer(eng, out_v, a_v, b_v, w):
                eng.tensor_mul(
                    out_v,
                    a_v.unsqueeze(2).broadcast_to((128, 8, 8, w)),
                    b_v.unsqueeze(1).broadcast_to((128, 8, 8, w)),
                )

            b0 = 0
            for ci, ns in enumerate(CHUNKS):
                w = 4 * ns
                q0 = 4 * b0
                qs = slice(q0, q0 + w)
                # factor views for this chunk: [p, 8, w]
                fc = FX[:, SEB * b0:SEB * (b0 + ns)].rearrange(
                    "p (f e q) -> p f e q", f=4, e=8, q=w
                )
                fv = [fc[:, f] for f in range(4)]
                dq = _dve_q(ns)
                outer(nc.vector, A6v[:, :, :, qs], fv[0], fv[1], w)
                if dq > 0:
                    outer(nc.vector, B6v[:, :, :, q0:q0 + dq],
                          fv[2][:, :, :dq], fv[3][:, :, :dq], dq)
                if dq < w:
                    outer(nc.gpsimd, B6v[:, :, :, q0 + dq:q0 + w],
                          fv[2][:, :, dq:], fv[3][:, :, dq:], w - dq)

                # stage 1: per (sample, parity) accumulate over s
                # psum pt[p, lb*32 + k], p = j + 64*(i&1), k = i>>1
                pt = psum.tile([128, ns * KCH], f32, tag=f"ps{ci}")
                Bc = B6v[:, :, :, :].rearrange("p hi lo q -> p (hi lo) q")
                for lb in range(ns):
                    col = slice(lb * KCH, (lb + 1) * KCH)
                    for s in range(SP):
                        q = q0 + lb * SP + s
                        lhsT = Bc[:, :, q]
                        nc.tensor.matmul(
                            pt[0:64, col], lhsT, A6v[:, :, 0::2, q],
                            start=(s == 0), stop=(s == SP - 1),
                            skip_group_check=True,
                        )
                        nc.tensor.matmul(
                            pt[64:128, col], lhsT, A6v[:, :, 1::2, q],
                            start=(s == 0), stop=(s == SP - 1),
                            tile_position=(0, 64),
                            skip_group_check=True,
                        )

                # psum -> T8 (fp8e4) with the t-mean scale
                nc.scalar.activation(
                    T8v[:, :, b0:b0 + ns],
                    pt[:].rearrange("p (lb k) -> p k lb", lb=ns, k=KCH),
                    Act.Copy,
                    scale=1.0 / T,
                )
                b0 += ns
                if ci < len(CHUNKS) - 1:
                    warm(N_WARMG)

            # stage 2: fp8 DoubleRow, T8 stationary -> psum [b, class]
            SCv = SC[:].rearrange("p (k n) -> p k n", k=KCH, n=NPAD)
            op = psum.tile([B, CS], f32, tag="out")
            for c in range(KCH // 2):
                nc.tensor.matmul(
                    op[:],
                    T8v[:, 2 * c:2 * c + 2, :],
                    SCv[:, 2 * c:2 * c + 2, :CS],
                    start=(c == 0), stop=(c == KCH // 2 - 1),
                    perf_mode=mybir.MatmulPerfMode.DoubleRow,
                    skip_group_check=True,
                )
            osb = work.tile([B, CS], f32)
            nc.scalar.activation(osb[:], op[:], Act.Copy)
            nc.sync.dma_start(o_d[:], osb[:])

    nc.compile()
    return nc


_NC_CACHE = None


def _get_nc():
    global _NC_CACHE
    if _NC_CACHE is None:
        _NC_CACHE = _build_nc()
    return _NC_CACHE


def make_in_maps(x, cuts, leaf_score):
    xl = np.ascontiguousarray(x[-1], dtype=np.float32)  # [B, T, H]
    cut = cuts[:, 0].astype(np.float32)                 # [H]
    # t = p*4 + s ;  xp[p, b, s, f]
    xp = xl.reshape(B, 128, SP, H).transpose(1, 0, 2, 3)
    h = np.stack([xp, 2.0 * xp - cut], axis=-1)         # [p, b, s, f, 2]

    def k3(f0):
        a = h[..., f0, :].astype(np.float16)
        b_ = h[..., f0 + 1, :].astype(np.float16)
        c = h[..., f0 + 2, :].astype(np.float16)
        ab = (a[..., :, None] * b_[..., None, :]).reshape(128, B, SP, 4)
        return (ab[..., :, None].astype(np.float16)
                * c[..., None, :]).astype(np.float16).reshape(128, B, SP, 8)

    # fac[p, b, s, fac, 8] -> per chunk [p, fac, 8, (b s)]
    fac = np.stack([k3(0), k3(3), k3(6), k3(9)], axis=3)
    parts = []
    b0 = 0
    for ns in CHUNKS:
        blk = fac[:, b0:b0 + ns]                        # [p, ns, s, fac, 8]
        parts.append(blk.transpose(0, 3, 4, 1, 2).reshape(128, 32 * 4 * ns))
        b0 += ns
    fx = np.ascontiguousarray(np.concatenate(parts, axis=1), dtype=np.float16)

    # scores: fp8e4m3 [p, k, n] with n padded to 128
    s8 = np.zeros((NCORES, 128, KCH, NPAD), dtype=ml_dtypes.float8_e4m3)
    sl = leaf_score.astype(np.float32).reshape(KCH, 128, C)
    for m in range(NCORES):
        s8[m, :, :, :CS] = (
            sl[:, :, m * CS:(m + 1) * CS].transpose(1, 0, 2)
            .astype(ml_dtypes.float8_e4m3)
        )
    return [
        {"fx": fx, "s": np.ascontiguousarray(s8[m].reshape(128, KCH * NPAD))}
        for m in range(NCORES)
    ]


def kernel(x, cuts, leaf_score):
    from concourse import bass_utils

    nc = _get_nc()
    in_maps = make_in_maps(x, cuts, leaf_score)
    res = bass_utils.run_bass_kernel_spmd(nc, in_maps, list(range(NCORES)))
    out = np.concatenate([res.results[m]["o"] for m in range(NCORES)], axis=1)
    return out.astype(np.float32)


# revision 16
# speedup vs baseline: 1.0745x; 1.0334x over previous
"""Trainium2 Bass kernel for NeuralDecisionTree (histogram_binning).

Math: out[b,c] = mean_t sum_l (prod_f h[b,t,f,bit_f(l)]) * score[l,c] with
h[...,0] = x, h[...,1] = 2x - cut_f  (D=1 -> W=[1,2], bias=[0,-cut]).

The 4096-leaf weight vector is kron(A6, B6) of two 64-leaf halves (features
0-5 -> i, 6-11 -> j, l = i*64 + j), each half kron(P8, Q8) of two 8-wide
3-feature factors.  The host precomputes the four 8-wide factors (PA, QA,
PB, QB) in f16; the device builds A6 = PA x QA and B6 = PB x QB (the
dominant elementwise work, split across DVE and GpSimd), reduces each
sample to a 64x64 second-moment matrix on the TensorEngine (psum partition
p = leaf%128, col = leaf//128), and contracts with leaf_score in fp8e4m3
DoubleRow matmuls (leaf_score stationary, psum out = [class, sample]).

Sharding: leaf_score is sharded by class columns (125 per core); x is
replicated and stage 1 recomputed per core.

Pipeline: factors upload in unequal sample-chunks (small first chunk for an
early DVE start, 1-sample last chunk to shorten the serial tail), scores
last on the same SP queue; per chunk the stage-1 matmuls and psum->sbuf
fp8 copies trail the DVE/Pool krons; output is written class-major
([128, B] f32) so the final DMA is a single cheap row-per-partition
transfer.  A dummy-matmul warmup stream holds the PE p-state at full clock
for the stage-1/stage-2 bursts.
"""

import numpy as np
import ml_dtypes

B, T, H = 16, 512, 12
NCORES = 8
C = 1000
CS = C // NCORES          # 125 classes per core
SP = 4                    # t = p*4 + s
BS = B * SP               # 64 (b-major: bs = b*4 + s)
KCH = 32                  # 128-leaf chunks
NPAD = 128                # padded class cols in the fp8 score tile
CHUNKS = (3, 4, 4, 4, 1)  # samples per factor-upload chunk
SEB = 4 * 8 * SP          # factor cols per sample (128)
N_WARM0 = 50              # PE warmup matmuls before stage-1
N_WARMG = 16              # PE keep-warm matmuls between chunk bursts


def _dve_q(s):
    """bs-cols of a chunk's B6 built on DVE (rest on GpSimd), balancing
    DVE (0.52 ns/el + 60/op) against GpSimd (1.98 ns/el + 95/op)."""
    return max(0, min(4 * s, round((149.5 * s - 10) / 64)))


def _build_nc():
    import concourse.bass as bass
    import concourse.bacc as bacc
    import concourse.mybir as mybir
    from concourse import tile

    f32 = mybir.dt.float32
    f16 = mybir.dt.float16
    f8 = mybir.dt.float8e4
    Act = mybir.ActivationFunctionType

    nc = bacc.Bacc(None, target_bir_lowering=False, debug=False)

    # factors: [p, sample, fac(PA,QA,PB,QB), 8, s]
    fx_d = nc.dram_tensor("fx", [128, B * SEB], f16, kind="ExternalInput")
    s_d = nc.dram_tensor("s", [128, KCH * NPAD], f8, kind="ExternalInput")
    o_d = nc.dram_tensor("o", [128, B], f32, kind="ExternalOutput")  # [class, b]

    with tile.TileContext(nc) as tc:
        with (
            tc.tile_pool(name="io", bufs=1) as io,
            tc.tile_pool(name="work", bufs=1) as work,
            tc.tile_pool(name="psum", bufs=1, space="PSUM") as psum,
        ):
            FX = io.tile([128, B * SEB], f16)
            SC = io.tile([128, KCH * NPAD], f8)
            b0 = 0
            for ns in CHUNKS:
                sl = slice(b0 * SEB, (b0 + ns) * SEB)
                nc.sync.dma_start(FX[:, sl], fx_d[:, sl])
                b0 += ns
            nc.sync.dma_start(SC[:], s_d[:])

            A6 = work.tile([128, 64 * BS], f16)
            B6 = work.tile([128, 64 * BS], f16)
            # [p, hi, lo, bs]  (bs = b*4 + s)
            A6v = A6[:].rearrange("p (hi lo q) -> p hi lo q", hi=8, lo=8, q=BS)
            B6v = B6[:].rearrange("p (hi lo q) -> p hi lo q", hi=8, lo=8, q=BS)

            T8 = work.tile([128, KCH * B], f8)  # cols: k*16 + b
            T8v = T8[:].rearrange("p (k b) -> p k b", k=KCH, b=B)

            # PE warmup stream: junk matmuls to ramp/hold the p-state
            dw = work.tile([128, 64], f16)
            nc.vector.memzero(dw[:])
            dp = psum.tile([64, 64], f32, tag="warm")

            def warm(n):
                for _ in range(n):
                    nc.tensor.matmul(
                        dp[:], dw[:], dw[:], start=True, stop=True,
                        skip_group_check=True,
                    )

            warm(N_WARM0)

            def outer(eng, out_v, a_v, b_v, w):
                eng.tensor_mul(
                    out_v,
                    a_v.unsqueeze(2).broadcast_to((128, 8, 8, w)),
                    b_v.unsqueeze(1).broadcast_to((128, 8, 8, w)),
                )

            b0 = 0
            for ci, ns in enumerate(CHUNKS):
                w = 4 * ns
                q0 = 4 * b0
                qs = slice(q0, q0 + w)
                # factor views for this chunk: [p, 8, w]
                fc = FX[:, SEB * b0:SEB * (b0 + ns)].rearrange(
                    "p (f e q) -> p f e q", f=4, e=8, q=w
                )
                fv = [fc[:, f] for f in range(4)]
                dq = _dve_q(ns)
                outer(nc.vector, A6v[:, :, :, qs], fv[0], fv[1], w)
                if dq > 0:
                    outer(nc.vector, B6v[:, :, :, q0:q0 + dq],
                          fv[2][:, :, :dq], fv[3][:, :, :dq], dq)
                if dq < w:
                    outer(nc.gpsimd, B6v[:, :, :, q0 + dq:q0 + w],
                          fv[2][:, :, dq:], fv[3][:, :, dq:], w - dq)

                # stage 1: per (sample, parity) accumulate over s
                # psum pt[p, lb*32 + k], p = j + 64*(i&1), k = i>>1
                pt = psum.tile([128, ns * KCH], f32, tag=f"ps{ci}")
                Bc = B6v[:, :, :, :].rearrange("p hi lo q -> p (hi lo) q")
                for lb in range(ns):
                    col = slice(lb * KCH, (lb + 1) * KCH)
                    for s in range(SP):
                        q = q0 + lb * SP + s
                        lhsT = Bc[:, :, q]
                        nc.tensor.matmul(
                            pt[0:64, col], lhsT, A6v[:, :, 0::2, q],
                            start=(s == 0), stop=(s == SP - 1),
                            skip_group_check=True,
                        )
                        nc.tensor.matmul(
                            pt[64:128, col], lhsT, A6v[:, :, 1::2, q],
                            start=(s == 0), stop=(s == SP - 1),
                            tile_position=(0, 64),
                            skip_group_check=True,
                        )

                # psum -> T8 (fp8e4) with the t-mean scale
                nc.scalar.activation(
                    T8v[:, :, b0:b0 + ns],
                    pt[:].rearrange("p (lb k) -> p k lb", lb=ns, k=KCH),
                    Act.Copy,
                    scale=1.0 / T,
                )
                b0 += ns
                if ci < len(CHUNKS) - 1:
                    warm(N_WARMG)

            # stage 2: fp8 DoubleRow, scores stationary -> psum [class, b]
            # (two class-groups so psum partition counts stay <= 64)
            SCv = SC[:].rearrange("p (k n) -> p k n", k=KCH, n=NPAD)
            osb = work.tile([128, B], f32)  # [class(pad 128), b]
            for g, (c0, cn) in enumerate(((0, 64), (64, CS - 64))):
                og = psum.tile([cn, B], f32, tag=f"out{g}")
                for c in range(KCH // 2):
                    nc.tensor.matmul(
                        og[:],
                        SCv[:, 2 * c:2 * c + 2, c0:c0 + cn],
                        T8v[:, 2 * c:2 * c + 2, :],
                        start=(c == 0), stop=(c == KCH // 2 - 1),
                        perf_mode=mybir.MatmulPerfMode.DoubleRow,
                        skip_group_check=True,
                    )
                nc.vector.tensor_copy(osb[c0:c0 + cn, :], og[:])
            nc.sync.dma_start(o_d[0:CS, :], osb[0:CS, :])

    nc.compile()
    return nc


_NC_CACHE = None


def _get_nc():
    global _NC_CACHE
    if _NC_CACHE is None:
        _NC_CACHE = _build_nc()
    return _NC_CACHE


def make_in_maps(x, cuts, leaf_score):
    xl = np.ascontiguousarray(x[-1], dtype=np.float32)  # [B, T, H]
    cut = cuts[:, 0].astype(np.float32)                 # [H]
    # t = p*4 + s ;  xp[p, b, s, f]
    xp = xl.reshape(B, 128, SP, H).transpose(1, 0, 2, 3)
    h = np.stack([xp, 2.0 * xp - cut], axis=-1)         # [p, b, s, f, 2]

    def k3(f0):
        a = h[..., f0, :].astype(np.float16)
        b_ = h[..., f0 + 1, :].astype(np.float16)
        c = h[..., f0 + 2, :].astype(np.float16)
        ab = (a[..., :, None] * b_[..., None, :]).reshape(128, B, SP, 4)
        return (ab[..., :, None].astype(np.float16)
                * c[..., None, :]).astype(np.float16).reshape(128, B, SP, 8)

    # fac[p, b, s, fac, 8] -> per chunk [p, fac, 8, (b s)]
    fac = np.stack([k3(0), k3(3), k3(6), k3(9)], axis=3)
    parts = []
    b0 = 0
    for ns in CHUNKS:
        blk = fac[:, b0:b0 + ns]                        # [p, ns, s, fac, 8]
        parts.append(blk.transpose(0, 3, 4, 1, 2).reshape(128, 32 * 4 * ns))
        b0 += ns
    fx = np.ascontiguousarray(np.concatenate(parts, axis=1), dtype=np.float16)

    # scores: fp8e4m3 [p, k, n] with n padded to 128
    s8 = np.zeros((NCORES, 128, KCH, NPAD), dtype=ml_dtypes.float8_e4m3)
    sl = leaf_score.astype(np.float32).reshape(KCH, 128, C)
    for m in range(NCORES):
        s8[m, :, :, :CS] = (
            sl[:, :, m * CS:(m + 1) * CS].transpose(1, 0, 2)
            .astype(ml_dtypes.float8_e4m3)
        )
    return [
        {"fx": fx, "s": np.ascontiguousarray(s8[m].reshape(128, KCH * NPAD))}
        for m in range(NCORES)
    ]


def kernel(x, cuts, leaf_score):
    from concourse import bass_utils

    nc = _get_nc()
    in_maps = make_in_maps(x, cuts, leaf_score)
    res = bass_utils.run_bass_kernel_spmd(nc, in_maps, list(range(NCORES)))
    out = np.concatenate(
        [res.results[m]["o"][:CS, :].T for m in range(NCORES)], axis=1
    )
    return out.astype(np.float32)
